# revision 16
# baseline (speedup 1.0000x reference)
"""Trainium2 Bass kernel for nn_CkyLinear: grouped-dequant linear.

reference: W_r = ((W_q - zero) * scale).reshape(4096, 4096); out = x @ W_r.T + bias
  x     [8, 2048, 4096] f32
  W_q   [64, 262144] int32 (u8 codes)
  scale [1, 262144] f32
  zero  [1, 262144] f32
  bias  [4096] f32

Sharding: tensor-parallel over output features, 8 cores x 512 features
(column-parallel linear; x replicated; the op's group layout makes the
scale/zero tables shared by all cores).

Per core: dequantize the W shard on-chip into a resident [4096, 512]
bf16 weight (sub in f32, then mul with bf16 store - avoids bf16
cancellation error), then stream bf16 x^T tiles and run bf16 matmuls
(lhsT = x^T tile [128i, 128bs] stationary, rhs = W tile [128i, 512o] moving,
psum [128bs, 512o] f32 accumulated over 32 k-tiles). Bias is added by DVE
during PSUM->SBUF eviction; output is stored bf16 and upcast on host.

bf16 rationale: TRN2 PE streams 1 elem/cell/cycle for f32 AND bf16, so
the matmul roofline (~874 us/core here) is dtype-independent - but per-NC
HBM is ~358 GB/s, and f32 x (256 MiB replicated) put DMA at 86% busy,
leaking into PE stalls. bf16 halves x traffic and enables FWL.

Startup choreography (the kernel's only non-steady phase):
- ~7 us fixed engine prologue, then every early byte fights for HBM:
  x tiles 0/1 (4 MiB), codes (2 MiB), tables (1 MiB bf16).
- Dequant runs as 2 ops per 4-k-tile chunk ([128, 4, 512], amortizing the
  DVE fixed cost), DVE on 6 chunks / GpSimd on chunks {3, 6} so chunk
  completion order tracks the PE's consumption order.
- x-tiles 0+1 are processed jointly, k-outer across 4 psum banks, so each
  weight chunk is consumed at 1/4 the k-inner rate; their DMAs are split
  into kt-halves (x0a, x1a, x0b, x1b) so matmuls can start ~3 us earlier.
- A short warmup matmul burst on a memset tile pre-warms the PE HAM clock
  gate before the first real matmul.
"""
import sys

if "/opt/trn_rl_repo" not in sys.path:
    sys.path.insert(0, "/opt/trn_rl_repo")

import ml_dtypes
import numpy as np

import concourse.bass as bass
import concourse.tile as tile
from concourse import bacc, mybir
from concourse.bass_utils import run_bass_kernel_spmd

B, S, IN_F, OUT_F, GROUP = 8, 2048, 4096, 4096, 64
BS = B * S  # 16384
N_CORES = 8
O_SHARD = OUT_F // N_CORES  # 512
KT = IN_F // 128  # 32 k-tiles
BSB = 256  # bs columns per x tile (2 matmul groups of 128)
N_BST = BS // BSB  # 64
P = 128
KCH = 8  # dequant chunks (one DVE/GpSimd op pair each)
KPC = KT // KCH  # 4 k-tiles per chunk

GPS_CH = (3, 6)  # chunks dequantized by GpSimd; DVE takes the rest

_CACHED_NC = None


def _build():
    nc = bacc.Bacc(trn_type="TRN2", target_bir_lowering=False, debug=False)
    f32 = mybir.dt.float32
    bf16 = mybir.dt.bfloat16

    xt = nc.dram_tensor("xt", [N_BST * P, KT * BSB], bf16, kind="ExternalInput").ap()
    # partition-major weight codes / tables: row p holds [kt, o] / [kt, h]
    wq = nc.dram_tensor("wq", [P, KT * O_SHARD], mybir.dt.uint8, kind="ExternalInput").ap()
    scl = nc.dram_tensor("scl", [P, KT * GROUP], bf16, kind="ExternalInput").ap()
    zs = nc.dram_tensor("zs", [P, KT * GROUP], bf16, kind="ExternalInput").ap()
    bias_b = nc.dram_tensor("bias_b", [P, O_SHARD], f32, kind="ExternalInput").ap()
    out = nc.dram_tensor("out", [BS, O_SHARD], bf16, kind="ExternalOutput").ap()

    xt3 = xt.rearrange("(t p) f -> t p f", p=P)  # [64, 128, 8192]
    wq3 = wq.rearrange("p (c f) -> p c f", c=KCH)
    scl3 = scl.rearrange("p (c f) -> p c f", c=KCH)
    zs3 = zs.rearrange("p (c f) -> p c f", c=KCH)
    out3 = out.rearrange("(t h b) o -> t h b o", h=BSB // P, b=P)

    with tile.TileContext(nc) as tc:
        with (
            tc.tile_pool(name="wres", bufs=1) as wres_pool,
            tc.tile_pool(name="deq", bufs=8) as deq_pool,
            tc.tile_pool(name="tmpv", bufs=2) as tmpv_pool,
            tc.tile_pool(name="tmpg", bufs=2) as tmpg_pool,
            tc.tile_pool(name="bias", bufs=1) as bias_pool,
            tc.tile_pool(name="xin", bufs=4) as x_pool,
            tc.tile_pool(name="psum", bufs=8, space="PSUM") as psum_pool,
            tc.tile_pool(name="oev", bufs=6) as o_pool,
        ):
            # HAM warmup source: a memset tile needs no DMA. GpSimd's queue is
            # idle early, so the warmup matmuls can start right after the
            # prologue instead of behind DVE's queue.
            warm_sb = bias_pool.tile([P, O_SHARD], bf16, name="warm_sb")
            nc.gpsimd.memset(warm_sb[:], 0)

            # x tiles 0/1 stream on the sync ring in kt-quarters so the first
            # matmuls start as soon as x0's first quarter lands, and the
            # scalar ring's dequant chunks get a fair share of early HBM.
            xts = []
            for t in (0, 1):
                x_t = x_pool.tile([P, KT, BSB], bf16, name="x_t")
                xts.append(x_t)
            QK = KT // 4
            for q in range(4):
                for t in (0, 1):
                    nc.sync.dma_start(
                        xts[t][:, q * QK : (q + 1) * QK, :],
                        xt3[t][:, q * QK * BSB : (q + 1) * QK * BSB].rearrange(
                            "p (kt b) -> p kt b", b=BSB
                        ),
                    )
            bias_sb = bias_pool.tile([P, O_SHARD], f32)
            nc.sync.dma_start(bias_sb[:], bias_b[:])

            # chunked fetch of dequant inputs (scalar/ACT HWDGE ring)
            wq_ch, sc_ch, zs_ch = [], [], []
            for c in range(KCH):
                wq_t = deq_pool.tile([P, KPC, O_SHARD], mybir.dt.uint8, name="wq_t")
                sc_t = deq_pool.tile([P, KPC, GROUP], bf16, name="sc_t")
                zs_t = deq_pool.tile([P, KPC, GROUP], bf16, name="zs_t")
                nc.scalar.dma_start(wq_t[:].rearrange("p k o -> p (k o)"), wq3[:, c])
                nc.scalar.dma_start(sc_t[:].rearrange("p k h -> p (k h)"), scl3[:, c])
                nc.scalar.dma_start(zs_t[:].rearrange("p k h -> p (k h)"), zs3[:, c])
                wq_ch.append(wq_t)
                sc_ch.append(sc_t)
                zs_ch.append(zs_t)

            # HAM warmup: garbage matmuls while the DMAs stream in. Results
            # land in a psum buffer the main loop recycles.
            warm_ps = psum_pool.tile([P, O_SHARD], f32, name="ps")
            for _ in range(16):
                nc.tensor.matmul(
                    warm_ps[:], warm_sb[:, :P], warm_sb[:], start=True, stop=True
                )

            # dequant: tmp = wq - zero (f32, exact); w = tmp * scale (bf16).
            # DVE runs batched ops (4 k-tiles per op pair, chunk 0 split in
            # half for earliest k=0 availability); GpSimd runs per-k-tile ops
            # so its chunks become consumable incrementally.
            w_ch = []
            for c in range(KCH):
                w_c = wres_pool.tile([P, KPC, O_SHARD], bf16, name=f"w_{c}")
                gps = c in GPS_CH
                if gps:
                    spans = [(j, j + 1) for j in range(KPC)]
                elif c == 0:
                    spans = [(0, KPC // 2), (KPC // 2, KPC)]
                else:
                    spans = [(0, KPC)]
                for j0, j1 in spans:
                    kw = j1 - j0
                    tmp_t = (tmpg_pool if gps else tmpv_pool).tile(
                        [P, KPC, O_SHARD], f32, name="tmpg" if gps else "tmpv"
                    )
                    tmp4 = tmp_t[:, j0:j1].rearrange("p k (g h) -> p k g h", h=GROUP)
                    w_c4 = w_c[:, j0:j1].rearrange("p k (g h) -> p k g h", h=GROUP)
                    wq_c4 = wq_ch[c][:, j0:j1].rearrange(
                        "p k (g h) -> p k g h", h=GROUP
                    )
                    sc_b = sc_ch[c][:, j0:j1, None, :].broadcast_to(
                        [P, kw, O_SHARD // GROUP, GROUP]
                    )
                    zs_b = zs_ch[c][:, j0:j1, None, :].broadcast_to(
                        [P, kw, O_SHARD // GROUP, GROUP]
                    )
                    eng = nc.gpsimd if gps else nc.vector
                    eng.tensor_sub(tmp4, wq_c4, zs_b)
                    eng.tensor_mul(w_c4, tmp4, sc_b)
                w_ch.append(w_c)

            def w_k(k):
                c, j = divmod(k, KPC)
                return w_ch[c][:, j, :]

            # x-tiles 0+1 jointly, k-outer across 4 psum banks: each weight
            # chunk is consumed at 1/4 the k-inner rate, tracking dequant
            # production with minimal PE stall.
            pss = [psum_pool.tile([P, O_SHARD], f32, name="ps") for _ in range(4)]
            for k in range(KT):
                for t in (0, 1):
                    for h in range(BSB // P):
                        nc.tensor.matmul(
                            pss[2 * t + h][:],
                            xts[t][:, k, bass.ts(h, P)],
                            w_k(k),
                            start=(k == 0),
                            stop=(k == KT - 1),
                        )
            for t in (0, 1):
                for h in range(BSB // P):
                    ob = o_pool.tile([P, O_SHARD], bf16, name="ob")
                    nc.vector.tensor_add(ob[:], pss[2 * t + h][:], bias_sb[:])
                    nc.sync.dma_start(out3[t, h], ob[:])

            for t in range(2, N_BST):
                x_t = x_pool.tile([P, KT, BSB], bf16, name="x_t")
                dma_eng = nc.scalar if t % 2 == 0 else nc.sync
                dma_eng.dma_start(
                    x_t[:], xt3[t].rearrange("p (kt b) -> p kt b", b=BSB)
                )
                pss = [
                    psum_pool.tile([P, O_SHARD], f32, name="ps")
                    for _ in range(BSB // P)
                ]
                for k in range(KT):
                    for h in range(BSB // P):
                        nc.tensor.matmul(
                            pss[h][:],
                            x_t[:, k, bass.ts(h, P)],
                            w_k(k),
                            start=(k == 0),
                            stop=(k == KT - 1),
                        )
                for h in range(BSB // P):
                    ob = o_pool.tile([P, O_SHARD], bf16, name="ob")
                    nc.vector.tensor_add(ob[:], pss[h][:], bias_sb[:])
                    nc.sync.dma_start(out3[t, h], ob[:])
    nc.compile()
    return nc


def kernel(x, W_q, scale, zero, bias):
    global _CACHED_NC
    if _CACHED_NC is None:
        _CACHED_NC = _build()
    nc = _CACHED_NC

    x = np.asarray(x)
    W_q = np.asarray(W_q)
    scale = np.asarray(scale)
    zero = np.asarray(zero)
    bias = np.asarray(bias)

    # Host-side layout staging (sharding + transposes + dtype cast, no W
    # arithmetic). x[t*256+b, kt*128+p] -> xh[t*128+p, kt*256+b]
    xh = np.ascontiguousarray(
        x.reshape(N_BST, BSB, KT, P).transpose(0, 3, 2, 1).reshape(N_BST * P, KT * BSB)
    ).astype(ml_dtypes.bfloat16)
    w3 = W_q.astype(np.uint8).reshape(GROUP, GROUP, IN_F)  # [g, h, i]
    s2 = scale.astype(np.float32).reshape(GROUP, IN_F)  # [h, i]
    z2 = zero.astype(np.float32).reshape(GROUP, IN_F)  # [h, i]
    # tables partition-major: [i, h] -> [p, kt, h] -> [p, kt*h]
    sclT = np.ascontiguousarray(
        s2.T.reshape(KT, P, GROUP).transpose(1, 0, 2).reshape(P, KT * GROUP)
    ).astype(ml_dtypes.bfloat16)
    zsT = np.ascontiguousarray(
        z2.T.reshape(KT, P, GROUP).transpose(1, 0, 2).reshape(P, KT * GROUP)
    ).astype(ml_dtypes.bfloat16)

    in_maps = []
    for c in range(N_CORES):
        # codes [i, gl*64+h] -> partition-major [p, kt*(gl*64+h)]
        wq_c = (
            w3[N_CORES * c : N_CORES * (c + 1)]
            .transpose(2, 0, 1)
            .reshape(KT, P, O_SHARD)
            .transpose(1, 0, 2)
            .reshape(P, KT * O_SHARD)
        )
        wq_c = np.ascontiguousarray(wq_c)
        bias_c = bias[O_SHARD * c : O_SHARD * (c + 1)].astype(np.float32)
        bias_bc = np.ascontiguousarray(np.broadcast_to(bias_c, (P, O_SHARD)))
        in_maps.append(
            {"xt": xh, "wq": wq_c, "scl": sclT, "zs": zsT, "bias_b": bias_bc}
        )

    res = run_bass_kernel_spmd(nc, in_maps, core_ids=list(range(N_CORES)))
    out = np.concatenate(
        [res.results[c]["out"].astype(np.float32) for c in range(N_CORES)], axis=1
    )
    return out.reshape(B, S, OUT_F)


# revision 17
# speedup vs baseline: 1.0013x; 1.0013x over previous
"""Trainium2 Bass kernel for nn_CkyLinear: grouped-dequant linear.

reference: W_r = ((W_q - zero) * scale).reshape(4096, 4096); out = x @ W_r.T + bias
  x     [8, 2048, 4096] f32
  W_q   [64, 262144] int32 (u8 codes)
  scale [1, 262144] f32
  zero  [1, 262144] f32
  bias  [4096] f32

Sharding: tensor-parallel over output features, 8 cores x 512 features
(column-parallel linear; x replicated; the op's group layout makes the
scale/zero tables shared by all cores).

Per core: dequantize the W shard on-chip into a resident [4096, 512]
bf16 weight (sub in f32, then mul with bf16 store - avoids bf16
cancellation error), then stream bf16 x^T tiles and run bf16 matmuls
(lhsT = x^T tile [128i, 128bs] stationary, rhs = W tile [128i, 512o] moving,
psum [128bs, 512o] f32 accumulated over 32 k-tiles). Bias is added by DVE
during PSUM->SBUF eviction; output is stored bf16 and upcast on host.

bf16 rationale: TRN2 PE streams 1 elem/cell/cycle for f32 AND bf16, so
the matmul roofline (~874 us/core here) is dtype-independent - but per-NC
HBM is ~358 GB/s, and f32 x (256 MiB replicated) put DMA at 86% busy,
leaking into PE stalls. bf16 halves x traffic and enables FWL.

Startup choreography (the kernel's only non-steady phase):
- ~7 us fixed engine prologue, then every early byte fights for HBM:
  x tiles 0/1 (4 MiB), codes (2 MiB), tables (1 MiB bf16).
- Dequant runs as 2 ops per 4-k-tile chunk ([128, 4, 512], amortizing the
  DVE fixed cost), DVE on 6 chunks / GpSimd on chunks {3, 6} so chunk
  completion order tracks the PE's consumption order.
- x-tiles 0+1 are processed jointly, k-outer across 4 psum banks, so each
  weight chunk is consumed at 1/4 the k-inner rate; their DMAs are split
  into kt-halves (x0a, x1a, x0b, x1b) so matmuls can start ~3 us earlier.
- A short warmup matmul burst on a memset tile pre-warms the PE HAM clock
  gate before the first real matmul.
"""
import sys

if "/opt/trn_rl_repo" not in sys.path:
    sys.path.insert(0, "/opt/trn_rl_repo")

import ml_dtypes
import numpy as np

import concourse.bass as bass
import concourse.tile as tile
from concourse import bacc, mybir
from concourse.bass_utils import run_bass_kernel_spmd

B, S, IN_F, OUT_F, GROUP = 8, 2048, 4096, 4096, 64
BS = B * S  # 16384
N_CORES = 8
O_SHARD = OUT_F // N_CORES  # 512
KT = IN_F // 128  # 32 k-tiles
BSB = 256  # bs columns per x tile (2 matmul groups of 128)
N_BST = BS // BSB  # 64
P = 128
KCH = 8  # dequant chunks (one DVE/GpSimd op pair each)
KPC = KT // KCH  # 4 k-tiles per chunk

GPS_CH = (3, 6)  # chunks dequantized by GpSimd; DVE takes the rest

_CACHED_NC = None


def _build():
    nc = bacc.Bacc(trn_type="TRN2", target_bir_lowering=False, debug=False)
    f32 = mybir.dt.float32
    bf16 = mybir.dt.bfloat16

    xt = nc.dram_tensor("xt", [N_BST * P, KT * BSB], bf16, kind="ExternalInput").ap()
    # partition-major weight codes / tables: row p holds [kt, o] / [kt, h]
    wq = nc.dram_tensor("wq", [P, KT * O_SHARD], mybir.dt.uint8, kind="ExternalInput").ap()
    scl = nc.dram_tensor("scl", [P, KT * GROUP], bf16, kind="ExternalInput").ap()
    zs = nc.dram_tensor("zs", [P, KT * GROUP], bf16, kind="ExternalInput").ap()
    bias_b = nc.dram_tensor("bias_b", [P, O_SHARD], f32, kind="ExternalInput").ap()
    out = nc.dram_tensor("out", [BS, O_SHARD], bf16, kind="ExternalOutput").ap()

    xt3 = xt.rearrange("(t p) f -> t p f", p=P)  # [64, 128, 8192]
    wq3 = wq.rearrange("p (c f) -> p c f", c=KCH)
    scl3 = scl.rearrange("p (c f) -> p c f", c=KCH)
    zs3 = zs.rearrange("p (c f) -> p c f", c=KCH)
    out3 = out.rearrange("(t h b) o -> t h b o", h=BSB // P, b=P)

    with tile.TileContext(nc) as tc:
        with (
            tc.tile_pool(name="wres", bufs=1) as wres_pool,
            tc.tile_pool(name="deq", bufs=8) as deq_pool,
            tc.tile_pool(name="tmpv", bufs=2) as tmpv_pool,
            tc.tile_pool(name="tmpg", bufs=2) as tmpg_pool,
            tc.tile_pool(name="bias", bufs=1) as bias_pool,
            tc.tile_pool(name="xin", bufs=4) as x_pool,
            tc.tile_pool(name="psum", bufs=8, space="PSUM") as psum_pool,
            tc.tile_pool(name="oev", bufs=4) as o_pool,
        ):
            # HAM warmup source: a memset tile needs no DMA. GpSimd's queue is
            # idle early, so the warmup matmuls can start right after the
            # prologue instead of behind DVE's queue.
            warm_sb = bias_pool.tile([P, O_SHARD], bf16, name="warm_sb")
            nc.gpsimd.memset(warm_sb[:], 0)

            # x tiles 0/1 stream on the sync ring in kt-quarters so the first
            # matmuls start as soon as x0's first quarter lands, and the
            # scalar ring's dequant chunks get a fair share of early HBM.
            xts = []
            for t in (0, 1):
                x_t = x_pool.tile([P, KT, BSB], bf16, name="x_t")
                xts.append(x_t)
            QK = KT // 4
            for q in range(4):
                for t in (0, 1):
                    nc.sync.dma_start(
                        xts[t][:, q * QK : (q + 1) * QK, :],
                        xt3[t][:, q * QK * BSB : (q + 1) * QK * BSB].rearrange(
                            "p (kt b) -> p kt b", b=BSB
                        ),
                    )
            bias_sb = bias_pool.tile([P, O_SHARD], f32)
            nc.sync.dma_start(bias_sb[:], bias_b[:])

            # chunked fetch of dequant inputs (scalar/ACT HWDGE ring)
            wq_ch, sc_ch, zs_ch = [], [], []
            for c in range(KCH):
                wq_t = deq_pool.tile([P, KPC, O_SHARD], mybir.dt.uint8, name="wq_t")
                sc_t = deq_pool.tile([P, KPC, GROUP], bf16, name="sc_t")
                zs_t = deq_pool.tile([P, KPC, GROUP], bf16, name="zs_t")
                nc.scalar.dma_start(wq_t[:].rearrange("p k o -> p (k o)"), wq3[:, c])
                nc.scalar.dma_start(sc_t[:].rearrange("p k h -> p (k h)"), scl3[:, c])
                nc.scalar.dma_start(zs_t[:].rearrange("p k h -> p (k h)"), zs3[:, c])
                wq_ch.append(wq_t)
                sc_ch.append(sc_t)
                zs_ch.append(zs_t)

            # HAM warmup: garbage matmuls while the DMAs stream in. Results
            # land in a psum buffer the main loop recycles.
            warm_ps = psum_pool.tile([P, O_SHARD], f32, name="ps")
            for _ in range(6):
                nc.tensor.matmul(
                    warm_ps[:], warm_sb[:, :P], warm_sb[:], start=True, stop=True
                )

            # dequant: tmp = wq - zero (f32, exact); w = tmp * scale (bf16).
            # DVE runs batched ops (4 k-tiles per op pair, chunk 0 split in
            # half for earliest k=0 availability); GpSimd runs per-k-tile ops
            # so its chunks become consumable incrementally.
            w_ch = []
            for c in range(KCH):
                w_c = wres_pool.tile([P, KPC, O_SHARD], bf16, name=f"w_{c}")
                gps = c in GPS_CH
                if gps:
                    spans = [(j, j + 1) for j in range(KPC)]
                elif c == 0:
                    spans = [(0, KPC // 2), (KPC // 2, KPC)]
                else:
                    spans = [(0, KPC)]
                for j0, j1 in spans:
                    kw = j1 - j0
                    tmp_t = (tmpg_pool if gps else tmpv_pool).tile(
                        [P, KPC, O_SHARD], f32, name="tmpg" if gps else "tmpv"
                    )
                    tmp4 = tmp_t[:, j0:j1].rearrange("p k (g h) -> p k g h", h=GROUP)
                    w_c4 = w_c[:, j0:j1].rearrange("p k (g h) -> p k g h", h=GROUP)
                    wq_c4 = wq_ch[c][:, j0:j1].rearrange(
                        "p k (g h) -> p k g h", h=GROUP
                    )
                    sc_b = sc_ch[c][:, j0:j1, None, :].broadcast_to(
                        [P, kw, O_SHARD // GROUP, GROUP]
                    )
                    zs_b = zs_ch[c][:, j0:j1, None, :].broadcast_to(
                        [P, kw, O_SHARD // GROUP, GROUP]
                    )
                    eng = nc.gpsimd if gps else nc.vector
                    eng.tensor_sub(tmp4, wq_c4, zs_b)
                    eng.tensor_mul(w_c4, tmp4, sc_b)
                w_ch.append(w_c)

            def w_k(k):
                c, j = divmod(k, KPC)
                return w_ch[c][:, j, :]

            # x-tiles 0+1 jointly, k-outer across 4 psum banks: each weight
            # chunk is consumed at 1/4 the k-inner rate, tracking dequant
            # production with minimal PE stall.
            pss = [psum_pool.tile([P, O_SHARD], f32, name="ps") for _ in range(4)]
            for k in range(KT):
                for t in (0, 1):
                    for h in range(BSB // P):
                        nc.tensor.matmul(
                            pss[2 * t + h][:],
                            xts[t][:, k, bass.ts(h, P)],
                            w_k(k),
                            start=(k == 0),
                            stop=(k == KT - 1),
                        )
            for t in (0, 1):
                for h in range(BSB // P):
                    ob = o_pool.tile([P, O_SHARD], bf16, name="ob")
                    nc.vector.tensor_add(ob[:], pss[2 * t + h][:], bias_sb[:])
                    nc.sync.dma_start(out3[t, h], ob[:])

            for t in range(2, N_BST):
                x_t = x_pool.tile([P, KT, BSB], bf16, name="x_t")
                dma_eng = nc.scalar if t % 2 == 0 else nc.sync
                dma_eng.dma_start(
                    x_t[:], xt3[t].rearrange("p (kt b) -> p kt b", b=BSB)
                )
                pss = [
                    psum_pool.tile([P, O_SHARD], f32, name="ps")
                    for _ in range(BSB // P)
                ]
                for k in range(KT):
                    for h in range(BSB // P):
                        nc.tensor.matmul(
                            pss[h][:],
                            x_t[:, k, bass.ts(h, P)],
                            w_k(k),
                            start=(k == 0),
                            stop=(k == KT - 1),
                        )
                for h in range(BSB // P):
                    ob = o_pool.tile([P, O_SHARD], bf16, name="ob")
                    nc.vector.tensor_add(ob[:], pss[h][:], bias_sb[:])
                    nc.sync.dma_start(out3[t, h], ob[:])
    nc.compile()
    return nc


def kernel(x, W_q, scale, zero, bias):
    global _CACHED_NC
    if _CACHED_NC is None:
        _CACHED_NC = _build()
    nc = _CACHED_NC

    x = np.asarray(x)
    W_q = np.asarray(W_q)
    scale = np.asarray(scale)
    zero = np.asarray(zero)
    bias = np.asarray(bias)

    # Host-side layout staging (sharding + transposes + dtype cast, no W
    # arithmetic). x[t*256+b, kt*128+p] -> xh[t*128+p, kt*256+b]
    xh = np.ascontiguousarray(
        x.reshape(N_BST, BSB, KT, P).transpose(0, 3, 2, 1).reshape(N_BST * P, KT * BSB)
    ).astype(ml_dtypes.bfloat16)
    w3 = W_q.astype(np.uint8).reshape(GROUP, GROUP, IN_F)  # [g, h, i]
    s2 = scale.astype(np.float32).reshape(GROUP, IN_F)  # [h, i]
    z2 = zero.astype(np.float32).reshape(GROUP, IN_F)  # [h, i]
    # tables partition-major: [i, h] -> [p, kt, h] -> [p, kt*h]
    sclT = np.ascontiguousarray(
        s2.T.reshape(KT, P, GROUP).transpose(1, 0, 2).reshape(P, KT * GROUP)
    ).astype(ml_dtypes.bfloat16)
    zsT = np.ascontiguousarray(
        z2.T.reshape(KT, P, GROUP).transpose(1, 0, 2).reshape(P, KT * GROUP)
    ).astype(ml_dtypes.bfloat16)

    in_maps = []
    for c in range(N_CORES):
        # codes [i, gl*64+h] -> partition-major [p, kt*(gl*64+h)]
        wq_c = (
            w3[N_CORES * c : N_CORES * (c + 1)]
            .transpose(2, 0, 1)
            .reshape(KT, P, O_SHARD)
            .transpose(1, 0, 2)
            .reshape(P, KT * O_SHARD)
        )
        wq_c = np.ascontiguousarray(wq_c)
        bias_c = bias[O_SHARD * c : O_SHARD * (c + 1)].astype(np.float32)
        bias_bc = np.ascontiguousarray(np.broadcast_to(bias_c, (P, O_SHARD)))
        in_maps.append(
            {"xt": xh, "wq": wq_c, "scl": sclT, "zs": zsT, "bias_b": bias_bc}
        )

    res = run_bass_kernel_spmd(nc, in_maps, core_ids=list(range(N_CORES)))
    out = np.concatenate(
        [res.results[c]["out"].astype(np.float32) for c in range(N_CORES)], axis=1
    )
    return out.reshape(B, S, OUT_F)
